# revision 24
# baseline (speedup 1.0000x reference)
"""CrossLayerTranscoder Trainium2 kernel.

Shards the d_transcoder (feature) axis across 8 NeuronCores (768 features
per layer per core).  Each core encodes its feature shard for all 6 layers
(acts kept feature-major on-chip), then decodes partial reconstructions for
every layer j accumulating over source layers i <= j.  The feature-shard
all-reduce is done on the host when unsharding (partials summed + b_dec).

All device inputs are pre-swizzled on the host so that every DMA is a
[128, 4608] tile whose per-partition row is 18KB contiguous in DRAM.
"""

import numpy as np

import concourse.bass as bass
import concourse.mybir as mybir
from concourse.bass import ts
from concourse.tile import TileContext
from concourse.bass_utils import run_bass_kernel_spmd

L = 6            # layers
T = 128          # tokens
D = 768          # d_model
DT = 6144        # d_transcoder
N_CORES = 8
F = DT // N_CORES   # features per layer per core = 768
KD = D // 128       # d_model chunks of 128 = 6
KF = F // 128       # feature chunks of 128 = 6
F2 = F // 2         # feature half (encode ring split) = 384
# decode pairs in j-outer order (only upper triangle j >= i is nonzero)
PAIRS = [(i, j) for j in range(L) for i in range(j + 1)]
PAIR_IDX = {p: n for n, p in enumerate(PAIRS)}

F32 = mybir.dt.float32
BF16 = mybir.dt.bfloat16
U8 = mybir.dt.uint8
FP8E3 = mybir.dt.float8e3

# weight/activation dtype on device ("f32" or "bf16"); PSUM accum is always f32
WEIGHT_DTYPE = "bf16"
# W_enc/W_dec ride as fp8 e3m4 (1-3-4, bias 3, max normal 15.5), pre-scaled
# by WD_SCALE on the host.  S=192 keeps both the denormal mass and the clip
# mass negligible (HW-verified: PE preserves e3m4 denormals; measured err
# tracks the ml_dtypes sim).  b_enc is pre-scaled by S so acts carry the
# encode x S factor in bf16; the decode PSUM->SBUF copy unscales by 1/S^2.
# End-to-end rel-err ~1.8e-2 vs the 2e-2 gate.
WD_SCALE = 192.0
WD_MAXN = 15.5
# matmul compute mode: "f32" (2-pass LOW_HIGH, exact) or "f32r" (single-pass,
# reduced-precision multiply) — f32r bitcasts the same fp32 bytes at the
# matmul call sites only.
MM_MODE = "f32"


def _mm_ap(ap):
    if MM_MODE == "f32r" and ap.dtype == F32:
        return ap.bitcast(mybir.dt.float32r)
    return ap


def _split_multiwaits(nc):
    """This container's walrus rejects >1 sync-wait per instruction; split
    extra waits onto same-engine NOPs inserted immediately before."""
    for fn in nc.m.functions:
        for bb in fn.blocks:
            new = []
            for ins in bb.instructions:
                si = ins.sync_info
                if si is not None and si.on_wait and len(si.on_wait) > 1:
                    waits = list(si.on_wait)
                    for w in waits[:-1]:
                        nop = mybir.InstNoOp(
                            name=nc.get_next_instruction_name(),
                            engine=ins.engine,
                            ins=[],
                            outs=[],
                            sync_info=mybir.SyncInfo(on_wait=[w], on_update=[]),
                        )
                        new.append(nop)
                    ins.sync_info = mybir.SyncInfo(
                        on_wait=[waits[-1]], on_update=list(si.on_update or [])
                    )
                new.append(ins)
            bb.instructions = new


def _build_nc(wdt):
    nc = bass.Bass()
    xt_d = nc.dram_tensor("xt", [L, 128, KD, T], wdt, kind="ExternalInput")
    # encode/decode weights pre-split into two d-halves, one per HWDGE ring
    we_d = nc.dram_tensor("we", [L, 2, 128, KD, F2], U8, kind="ExternalInput")
    wd_d = nc.dram_tensor(
        "wd", [len(PAIRS), 2, 128, KF, 384], U8, kind="ExternalInput"
    )
    be_d = nc.dram_tensor("be", [128, L, KF], F32, kind="ExternalInput")
    out_d = nc.dram_tensor("out", [L, 128, D], wdt, kind="ExternalOutput")

    # Every bulk load is split into two equal halves, half h pinned to ring h
    # (SP / Activation HWDGE).  Both rings then carry identical byte streams
    # and advance in lockstep, so weight-tile arrival order exactly matches
    # the PE's program order (a ready tile can otherwise sit behind a
    # not-yet-arrived one in the in-order PE queue).
    # Stores go off-ring (gpsimd SWDGE): they depend on compute, and on a
    # HWDGE ring they would head-of-line-block the weight loads behind them.
    rings = [nc.sync, nc.scalar]

    with TileContext(nc) as tc:
        with (
            tc.tile_pool(name="const", bufs=1) as cpool,
            tc.tile_pool(name="we", bufs=3) as wepool,
            tc.tile_pool(name="wd", bufs=10) as wdpool,
            tc.tile_pool(name="pse", bufs=4, space="PSUM") as pse,
            tc.tile_pool(name="psd", bufs=4, space="PSUM") as psd,
        ):
            X = cpool.tile([128, L, KD, T], wdt, tag="x")
            BE = cpool.tile([128, L, KF], F32, tag="be")
            A = cpool.tile([128, L, KF, T], wdt, tag="acts")
            # x + b_enc ride the otherwise-idle SWDGE ring so the two HWDGE
            # rings carry nothing but weight streams (an x load queued ahead
            # of a w_enc layer otherwise delays that layer's matmuls)
            nc.gpsimd.dma_start(out=X[:, 0, :, :], in_=xt_d[0])
            nc.gpsimd.dma_start(out=BE[:], in_=be_d[:])
            for l in range(1, L):
                nc.gpsimd.dma_start(out=X[:, l, :, :], in_=xt_d[l])

            # ---- encode: acts[f, t] = relu(W_enc^T-chunks @ x^T + b_enc)
            for l in range(L):
                we = wepool.tile([128, 2, KD, F2], U8, tag="we")
                for h in range(2):
                    rings[h].dma_start(out=we[:, h], in_=we_d[l, h])
                for ft in range(KF):
                    h, f2 = divmod(ft, KF // 2)
                    ps = pse.tile([128, T], F32, tag="pse")
                    for kd in range(KD):
                        nc.tensor.matmul(
                            ps[:],
                            we[:, h, kd, ts(f2, 128)].bitcast(FP8E3),
                            _mm_ap(X[:, l, kd, :]),
                            start=(kd == 0),
                            stop=(kd == KD - 1),
                        )
                    # relu(ps + b_enc) on DVE — keeps ScalarE (and its
                    # activation-table preamble DMA) out of the kernel
                    nc.vector.tensor_scalar(
                        out=A[:, l, ft, :],
                        in0=ps[:],
                        scalar1=BE[:, l, ts(ft, 1)],
                        scalar2=0.0,
                        op0=mybir.AluOpType.add,
                        op1=mybir.AluOpType.max,
                    )

            # ---- decode: recon[j][t, d] = sum_{i<=j} acts_i^T-chunks @ W_dec[i,j]
            # Partials are stored as bf16 (halves write traffic; the 8-core
            # host-side reduction keeps the extra error ~0.4% in quadrature).
            OUT = cpool.tile([128, L, D], wdt, tag="out")
            for j in range(L):
                ps0 = psd.tile([128, 384], F32, tag="psd")
                ps1 = psd.tile([128, 384], F32, tag="psd")
                for i in range(j + 1):
                    wd = wdpool.tile([128, 2, KF, 384], U8, tag="wd")
                    n = PAIR_IDX[(i, j)]
                    for h in range(2):
                        rings[h].dma_start(out=wd[:, h], in_=wd_d[n, h])
                    # d-half 0 for all kf, then d-half 1 — lets the ps0
                    # accumulation close earlier so its copy/DMA overlaps
                    for kf in range(KF):
                        nc.tensor.matmul(
                            ps0[:],
                            _mm_ap(A[:, i, kf, :]),
                            wd[:, 0, kf, :].bitcast(FP8E3),
                            start=(i == 0 and kf == 0),
                            stop=(i == j and kf == KF - 1),
                        )
                    for kf in range(KF):
                        nc.tensor.matmul(
                            ps1[:],
                            _mm_ap(A[:, i, kf, :]),
                            wd[:, 1, kf, :].bitcast(FP8E3),
                            start=(i == 0 and kf == 0),
                            stop=(i == j and kf == KF - 1),
                        )
                # PSUM->SBUF copy with the fp8 pre-scale folded in
                nc.vector.tensor_scalar(
                    out=OUT[:, j, 0:384], in0=ps0[:],
                    scalar1=1.0 / (WD_SCALE * WD_SCALE), scalar2=None,
                    op0=mybir.AluOpType.mult,
                )
                nc.vector.tensor_scalar(
                    out=OUT[:, j, 384:768], in0=ps1[:],
                    scalar1=1.0 / (WD_SCALE * WD_SCALE), scalar2=None,
                    op0=mybir.AluOpType.mult,
                )
                if j < L - 1:
                    # off-ring store, overlapped with the remaining stream
                    nc.gpsimd.dma_start(out=out_d[j], in_=OUT[:, j, :])
                else:
                    # final store: all loads have drained by now, so the
                    # fast RTL HWDGE path beats Q7 descriptor-gen
                    nc.sync.dma_start(out=out_d[j], in_=OUT[:, j, :])

    _split_multiwaits(nc)
    return nc


_NC_CACHE = {}


def _get_nc(wdt):
    key = str(wdt)
    if key not in _NC_CACHE:
        _NC_CACHE[key] = _build_nc(wdt)
    return _NC_CACHE[key]


def _np_wdt():
    if WEIGHT_DTYPE == "bf16":
        import ml_dtypes

        return np.dtype(ml_dtypes.bfloat16)
    return np.dtype(np.float32)


def _shard_inputs(x, W_enc, b_enc):
    """Host-side pre-swizzle into per-core DMA-friendly layouts."""
    npdt = _np_wdt()
    # xt[l, p, kd, t] = x[l, t, kd*128+p] — same on every core
    xt = np.ascontiguousarray(
        x.transpose(2, 0, 1).reshape(KD, 128, L, T).transpose(2, 1, 0, 3)
    ).astype(npdt)
    import ml_dtypes

    e3m4 = np.dtype(ml_dtypes.float8_e3m4)
    in_maps = []
    for c in range(N_CORES):
        fs = c * F
        w = np.clip(W_enc[:, fs : fs + F, :] * WD_SCALE, -WD_MAXN, WD_MAXN)
        # we[l, h, p, kd, f2] = q(S * W_enc[l, fs+h*F2+f2, kd*128+p])
        we = (
            np.ascontiguousarray(
                w.transpose(0, 2, 1)                # [L, D, F]
                .reshape(L, KD, 128, 2, F2)
                .transpose(0, 3, 2, 1, 4)           # [L, 2, 128, KD, F2]
            )
            .astype(e3m4)
            .view(np.uint8)
        )
        # b_enc pre-scaled by S so acts carry the encode x S factor
        be = np.ascontiguousarray(
            b_enc[:, fs : fs + F].reshape(L, KF, 128).transpose(2, 0, 1)
        ).astype(np.float32) * WD_SCALE
        in_maps.append({"xt": xt, "we": we, "be": be})
    return in_maps


def _shard_wdec(W_dec):
    import ml_dtypes

    e3m4 = np.dtype(ml_dtypes.float8_e3m4)
    shards = []
    for c in range(N_CORES):
        fs = c * F
        wd = np.empty((len(PAIRS), 2, 128, KF, 384), dtype=np.uint8)
        for n, (i, j) in enumerate(PAIRS):
            blk = np.clip(
                W_dec[i, j, fs : fs + F, :] * WD_SCALE, -WD_MAXN, WD_MAXN
            )  # [F, D]
            # wd[n, h, p, kf, d] = q(blk)[kf*128+p, h*384+d]
            wd[n] = (
                blk.reshape(KF, 128, 2, 384)
                .transpose(2, 1, 0, 3)
                .astype(e3m4)
                .view(np.uint8)
            )
        shards.append(wd)
    return shards


def kernel(x, W_enc, b_enc, b_dec, W_dec, dec_mask=None, **_unused):
    x = np.asarray(x, dtype=np.float32)
    W_enc = np.asarray(W_enc, dtype=np.float32)
    b_enc = np.asarray(b_enc, dtype=np.float32)
    b_dec = np.asarray(b_dec, dtype=np.float32)
    W_dec = np.asarray(W_dec, dtype=np.float32)

    wdt = BF16 if WEIGHT_DTYPE == "bf16" else F32
    nc = _get_nc(wdt)

    in_maps = _shard_inputs(x, W_enc, b_enc)
    wd_shards = _shard_wdec(W_dec)
    for c in range(N_CORES):
        in_maps[c]["wd"] = wd_shards[c]

    res = run_bass_kernel_spmd(nc, in_maps, core_ids=list(range(N_CORES)))

    # host-side all-reduce over feature shards + decoder bias
    recon = np.zeros((L, T, D), dtype=np.float32)
    for c in range(N_CORES):
        recon += np.asarray(res.results[c]["out"]).astype(np.float32)
    recon += b_dec[:, None, :]
    return recon

